# revision 18
# baseline (speedup 1.0000x reference)
"""Trainium2 Bass kernel for nn_LocalNetwork (avgpool3d -> 3x LocallyConnected1D -> upsample3d).

Sharding: pure data parallelism — batch 256 split as 32 per core across 8 cores.

Per-core layout (B_loc=32 batches, 4 load groups of 8, conv pairs of 2 groups):
  partition p = (bl, dslice)  [8 x 15 = 120 partitions]
  Every DMA descriptor covers a 32KB-contiguous DRAM run (one (h,w) slice).

  - avg-pool over (ws) then (hs): 3+3 tensor_tensor adds (DVE runs TT at
    ~2.4 elem/cycle vs 1 for tensor_reduce)
  - depth pool + depth-conv taps: matmuls [120 -> 40] with 0/(1/48)
    matrices (fuses the /48 mean scale and the +/-1 depth shifts)
  - two load-groups pair into [80, 512] conv tiles so the ~680ns/instr
    DVE overhead is amortized over 16 batches
  - upsample: relu + h-expand into ws=0 lanes of U (scalar engine),
    w-expand ws=1..3 in place (vector, broadcast src); depth x3
    replication is FREE — three store DMAs per group read the same
    [40, 8192] partition-slice of U into interleaved DRAM depth slices.
  - loads issue on the sync HWDGE queue, stores on the scalar (Act)
    HWDGE queue: a store waiting for compute must not head-of-line
    block later loads.
"""

import numpy as np

import concourse.bass as bass
import concourse.mybir as mybir
from concourse import bacc
from concourse.bass_utils import run_bass_kernel_spmd
from concourse.tile import TileContext

F32 = mybir.dt.float32
ADD = mybir.AluOpType.add
MULT = mybir.AluOpType.mult
RELU = mybir.ActivationFunctionType.Relu

N_CORES = 8
B = 256
B_CORE = 32          # batches per core
G = 4                # load groups per core
B_GRP = 8            # batches per group
CORE_ELEMS = B_CORE * 15 * 64 * 128  # 3,932,160
BSTRIDE = 15 * 64 * 128              # 122,880
SLICE = 64 * 128                     # 8192 elems = one (h,w) plane = 32KB


def _pack_consts(w_depth, b_depth, w_lon, b_lon, w_lat, b_lat):
    """Returns (mm [120,128] f32, wts [104,6144] f32).

    mm: three matmul lhsT tiles [120,40] (cols 0:40 dn / 40:80 mid / 80:120 up)
        out[q=(bl,dp), f] = sum_p lhsT[p=(bl,dsl), q] * P2[p, f]
        coefficient 1/48 folds the avg-pool mean.
    wts: 12 x [40,512] conv weight/bias tiles, p=(bl,dp), f=(ho,wo).
    """
    mm = np.zeros((120, 128), np.float32)
    for bl in range(8):
        for dsl in range(15):
            p = bl * 15 + dsl
            grp = dsl // 3
            for col0, dp in ((0, grp + 1), (40, grp), (80, grp - 1)):
                if 0 <= dp <= 4:
                    mm[p, col0 + bl * 5 + dp] = 1.0 / 48.0

    dp = np.arange(5)[:, None, None]
    ho = np.arange(16)[None, :, None]
    wo = np.arange(32)[None, None, :]
    ld = wo * 112 + ho * 7 + (dp + 1)     # depth seq index (5,16,32)
    ll = dp * 544 + ho * 34 + (wo + 1)    # lon
    lt = dp * 576 + wo * 18 + (ho + 1)    # lat

    def tile(vec, idx):
        t = np.broadcast_to(np.asarray(vec)[idx][None], (8, 5, 16, 32))
        return t.reshape(40, 512)

    cols = []
    for j in range(3):
        cols.append(tile(np.asarray(w_depth)[:, j], ld))
    cols.append(tile(b_depth, ld))
    for j in range(3):
        cols.append(tile(np.asarray(w_lon)[:, j], ll))
    cols.append(tile(b_lon, ll))
    for j in range(3):
        cols.append(tile(np.asarray(w_lat)[:, j], lt))
    cols.append(tile(b_lat, lt))
    wts40 = np.concatenate(cols, axis=1)
    wts = np.zeros((104, 6144), np.float32)
    wts[0:40] = wts40
    wts[64:104] = wts40
    return mm, np.ascontiguousarray(wts)


def build_nc(reps: int = 1) -> bass.Bass:
    nc = bacc.Bacc("TRN2", target_bir_lowering=False, debug=False)
    x = nc.dram_tensor("x", [CORE_ELEMS], F32, kind="ExternalInput")
    mmc = nc.dram_tensor("mm", [120, 128], F32, kind="ExternalInput")
    wtc = nc.dram_tensor("wts", [104, 6144], F32, kind="ExternalInput")
    y = nc.dram_tensor("y", [CORE_ELEMS], F32, kind="ExternalOutput")

    with TileContext(nc) as tc:
        with (
            tc.tile_pool(name="cpool", bufs=1) as cpool,
            tc.tile_pool(name="inp", bufs=2) as inp,
            tc.tile_pool(name="outp", bufs=2) as outp,
            tc.tile_pool(name="work", bufs=2) as work,
            tc.tile_pool(name="psum", bufs=2, space="PSUM") as psum,
        ):
            MM = cpool.tile([120, 128], F32)
            WT = cpool.tile([104, 6144], F32)
            nc.sync.dma_start(MM[:], mmc[:])
            nc.sync.dma_start(WT[:], wtc[:])
            w = lambda i: WT[:, i * 512:(i + 1) * 512]
            wd0, wd1, wd2, bd = (w(i) for i in range(4))
            vl0, vl1, vl2, blon = (w(i) for i in range(4, 8))
            ul0, ul1, ul2, blat = (w(i) for i in range(8, 12))

            state = {}

            def load(g, pieces=2):
                # split loads so the first piece's pool reduce overlaps the
                # rest of the transfer (and the first group primes fast)
                off = (g % G) * B_GRP * BSTRIDE
                X = inp.tile([120, SLICE], F32)
                step = SLICE // pieces
                for c in range(pieces):
                    nc.sync.dma_start(
                        X[:, c * step:(c + 1) * step],
                        bass.AP(x, off + c * step,
                                [[BSTRIDE, 8], [SLICE, 15], [1, step]]))
                state[g] = X

            def pool(g):
                X = state.pop(g)
                # h,w avg-pool (sum): fused reduce over (hs, ws), one per
                # load half (ho 0:8 | 8:16)
                P2 = work.tile([120, 512], F32)
                for c in range(2):
                    nc.vector.tensor_reduce(
                        P2[:, c * 256:(c + 1) * 256]
                            .rearrange("p (ho wo) -> p ho wo", ho=8),
                        X[:, c * 4096:(c + 1) * 4096]
                            .rearrange("p (ho hs wo ws) -> p ho wo hs ws",
                                       ho=8, hs=4, wo=32, ws=4),
                        mybir.AxisListType.XY, ADD)
                state[("P2", g)] = P2

            def mm(g):
                # depth pool (/48) + conv taps; pair half a at partitions
                # 0:40, half b at 64:104 (matmul out base must be 0/32/64)
                if (g % G) >= 2:
                    # single group -> [40, 512] PSUM at base 0
                    Sdn = psum.tile([40, 512], F32)
                    S0 = psum.tile([40, 512], F32)
                    Sup = psum.tile([40, 512], F32)
                    P2 = state.pop(("P2", g))
                    nc.tensor.matmul(Sdn[:], MM[:, 0:40], P2[:], start=True, stop=True)
                    nc.tensor.matmul(S0[:], MM[:, 40:80], P2[:], start=True, stop=True)
                    nc.tensor.matmul(Sup[:], MM[:, 80:120], P2[:], start=True, stop=True)
                    state[("S1", g)] = (Sdn, S0, Sup)
                    return
                k, half = divmod(g, 2)
                if half == 0:
                    Sdn = psum.tile([104, 512], F32)
                    S0 = psum.tile([104, 512], F32)
                    Sup = psum.tile([104, 512], F32)
                    state[("S", k)] = (Sdn, S0, Sup)
                else:
                    Sdn, S0, Sup = state[("S", k)]
                P2 = state.pop(("P2", g))
                sl = slice(64 * half, 64 * half + 40)
                nc.tensor.matmul(Sdn[sl], MM[:, 0:40], P2[:], start=True, stop=True)
                nc.tensor.matmul(S0[sl], MM[:, 40:80], P2[:], start=True, stop=True)
                nc.tensor.matmul(Sup[sl], MM[:, 80:120], P2[:], start=True, stop=True)

            def conv_store_pair(k):
                ga, gb = 2 * k, 2 * k + 1
                Sdn, S0, Sup = state.pop(("S", k))
                # depth conv: 3 independent mults, then the add chain
                m = work.tile([104, 512], F32)
                m2 = work.tile([104, 512], F32)
                m3 = work.tile([104, 512], F32)
                nc.vector.tensor_tensor(m[:], wd0, Sdn[:], MULT)
                nc.vector.tensor_tensor(m2[:], wd1, S0[:], MULT)
                nc.vector.tensor_tensor(m3[:], wd2, Sup[:], MULT)
                nc.vector.tensor_tensor(m[:], m[:], m2[:], ADD)
                nc.vector.tensor_tensor(m[:], m[:], m3[:], ADD)
                nc.vector.tensor_tensor(m[:], m[:], bd, ADD)
                # relu into lon-padded tile Ydp[p, ho*34 + (wo+1)]
                Ydp = work.tile([104, 544], F32)
                Ydpv = Ydp[:].rearrange("p (ho wp) -> p ho wp", ho=16, wp=34)
                nc.gpsimd.memset(Ydpv[:, :, 0], 0)
                nc.gpsimd.memset(Ydpv[:, :, 33], 0)
                nc.vector.tensor_scalar_max(
                    Ydpv[:, :, 1:33],
                    m[:].rearrange("p (ho wo) -> p ho wo", ho=16), 0.0)

                # lon conv (along wo, free axis)
                m3v = m3[:].rearrange("p (ho wo) -> p ho wo", ho=16)
                mv = m[:].rearrange("p (ho wo) -> p ho wo", ho=16)
                m2v = m2[:].rearrange("p (ho wo) -> p ho wo", ho=16)
                w3 = lambda t: t.rearrange("p (ho wo) -> p ho wo", ho=16)
                nc.vector.tensor_tensor(mv, w3(vl0), Ydpv[:, :, 0:32], MULT)
                nc.vector.tensor_tensor(m2v, w3(vl1), Ydpv[:, :, 1:33], MULT)
                nc.vector.tensor_tensor(m3v, w3(vl2), Ydpv[:, :, 2:34], MULT)
                nc.vector.tensor_tensor(mv, mv, m2v, ADD)
                nc.vector.tensor_tensor(mv, mv, m3v, ADD)
                nc.vector.tensor_tensor(mv, mv, w3(blon), ADD)
                # relu into lat-padded tile Ylp[p, (ho+1)*32 + wo]
                Ylp = work.tile([104, 576], F32)
                nc.gpsimd.memset(Ylp[:, 0:32], 0)
                nc.gpsimd.memset(Ylp[:, 544:576], 0)
                nc.vector.tensor_scalar_max(Ylp[:, 32:544], m[:], 0.0)

                # lat conv (along ho, free axis; contiguous slices)
                nc.vector.tensor_tensor(m[:], ul0, Ylp[:, 0:512], MULT)
                nc.vector.tensor_tensor(m2[:], ul1, Ylp[:, 32:544], MULT)
                nc.vector.tensor_tensor(m3[:], ul2, Ylp[:, 64:576], MULT)
                nc.vector.tensor_tensor(m[:], m[:], m2[:], ADD)
                nc.vector.tensor_tensor(m[:], m[:], m3[:], ADD)
                nc.vector.tensor_tensor(m[:], m[:], blat, ADD)

                # upsample: relu + h-expand (scalar), w-expand (vector)
                A = work.tile([104, 2048], F32)  # (ho, hs, wo)
                Av = A[:].rearrange("p (ho hs wo) -> p ho hs wo", ho=16, hs=4)
                mb = m[:].rearrange("p (ho wo) -> p ho wo", ho=16) \
                         .unsqueeze(2).broadcast_to([104, 16, 4, 32])
                nc.scalar.activation(Av, mb, RELU)
                U = outp.tile([104, SLICE], F32)  # (h, wo, ws)
                Uw = U[:].rearrange("p (h wo ws) -> p h wo ws", h=64, ws=4)
                Ab = A[:].rearrange("p (h wo) -> p h wo", h=64) \
                         .unsqueeze(3).broadcast_to([104, 64, 32, 4])
                nc.vector.tensor_scalar_add(Uw, Ab, 0.0)

                # stores alternate queues per di (all stores are emitted
                # after all loads, so no head-of-line blocking on q1)
                for half, g in enumerate((ga, gb)):
                    off = (g % G) * B_GRP * BSTRIDE
                    for di in range(3):
                        eng = nc.scalar if half == 0 else nc.sync
                        eng.dma_start(
                            bass.AP(y, off + di * SLICE,
                                    [[BSTRIDE, 8], [3 * SLICE, 5], [1, SLICE]]),
                            U[64 * half:64 * half + 40, :])

            def conv_store_single(g):
                Sdn, S0, Sup = state.pop(("S1", g))
                w40 = lambda i: WT[0:40, i * 512:(i + 1) * 512]
                m = work.tile([40, 512], F32)
                m2 = work.tile([40, 512], F32)
                m3 = work.tile([40, 512], F32)
                nc.vector.tensor_tensor(m[:], w40(0), Sdn[:], MULT)
                nc.vector.tensor_tensor(m2[:], w40(1), S0[:], MULT)
                nc.vector.tensor_tensor(m3[:], w40(2), Sup[:], MULT)
                nc.vector.tensor_tensor(m[:], m[:], m2[:], ADD)
                nc.vector.tensor_tensor(m[:], m[:], m3[:], ADD)
                nc.vector.tensor_tensor(m[:], m[:], w40(3), ADD)
                Ydp = work.tile([40, 544], F32)
                Ydpv = Ydp[:].rearrange("p (ho wp) -> p ho wp", ho=16, wp=34)
                nc.gpsimd.memset(Ydpv[:, :, 0], 0)
                nc.gpsimd.memset(Ydpv[:, :, 33], 0)
                nc.vector.tensor_scalar_max(
                    Ydpv[:, :, 1:33],
                    m[:].rearrange("p (ho wo) -> p ho wo", ho=16), 0.0)
                mv = m[:].rearrange("p (ho wo) -> p ho wo", ho=16)
                m2v = m2[:].rearrange("p (ho wo) -> p ho wo", ho=16)
                m3v = m3[:].rearrange("p (ho wo) -> p ho wo", ho=16)
                w340 = lambda i: w40(i).rearrange("p (ho wo) -> p ho wo", ho=16)
                nc.vector.tensor_tensor(mv, w340(4), Ydpv[:, :, 0:32], MULT)
                nc.vector.tensor_tensor(m2v, w340(5), Ydpv[:, :, 1:33], MULT)
                nc.vector.tensor_tensor(m3v, w340(6), Ydpv[:, :, 2:34], MULT)
                nc.vector.tensor_tensor(mv, mv, m2v, ADD)
                nc.vector.tensor_tensor(mv, mv, m3v, ADD)
                nc.vector.tensor_tensor(mv, mv, w340(7), ADD)
                Ylp = work.tile([40, 576], F32)
                nc.gpsimd.memset(Ylp[:, 0:32], 0)
                nc.gpsimd.memset(Ylp[:, 544:576], 0)
                nc.vector.tensor_scalar_max(Ylp[:, 32:544], m[:], 0.0)
                nc.vector.tensor_tensor(m[:], w40(8), Ylp[:, 0:512], MULT)
                nc.vector.tensor_tensor(m2[:], w40(9), Ylp[:, 32:544], MULT)
                nc.vector.tensor_tensor(m3[:], w40(10), Ylp[:, 64:576], MULT)
                nc.vector.tensor_tensor(m[:], m[:], m2[:], ADD)
                nc.vector.tensor_tensor(m[:], m[:], m3[:], ADD)
                nc.vector.tensor_tensor(m[:], m[:], w40(11), ADD)
                A = work.tile([40, 2048], F32)
                Av = A[:].rearrange("p (ho hs wo) -> p ho hs wo", ho=16, hs=4)
                mb = m[:].rearrange("p (ho wo) -> p ho wo", ho=16) \
                         .unsqueeze(2).broadcast_to([40, 16, 4, 32])
                nc.scalar.activation(Av, mb, RELU)
                U = outp.tile([40, SLICE], F32)
                Uw = U[:].rearrange("p (h wo ws) -> p h wo ws", h=64, ws=4)
                Ab = A[:].rearrange("p (h wo) -> p h wo", h=64) \
                         .unsqueeze(3).broadcast_to([40, 64, 32, 4])
                nc.vector.tensor_scalar_add(Uw, Ab, 0.0)
                off = (g % G) * B_GRP * BSTRIDE
                for di in range(3):
                    eng = nc.scalar if (g + di) % 2 else nc.sync
                    eng.dma_start(
                        bass.AP(y, off + di * SLICE,
                                [[BSTRIDE, 8], [3 * SLICE, 5], [1, SLICE]]),
                        U[:])

            # software-pipelined emission
            for r in range(reps):
                b = r * G
                load(b + 0, pieces=4)
                load(b + 1)
                pool(b + 0)
                mm(b + 0)
                load(b + 2)
                pool(b + 1)
                mm(b + 1)
                load(b + 3)
                conv_store_pair(b // 2 + 0)
                with tc.tile_wait_until(0.072 + r * 0.2):
                    pool(b + 2)
                    mm(b + 2)
                conv_store_single(b + 2)
                with tc.tile_wait_until(0.082 + r * 0.2):
                    pool(b + 3)
                    mm(b + 3)
                conv_store_single(b + 3)

    nc.compile()
    return nc


_NC_CACHE = {}


def _get_nc(reps: int = 1):
    if reps not in _NC_CACHE:
        _NC_CACHE[reps] = build_nc(reps)
    return _NC_CACHE[reps]


def kernel(x, w_depth, b_depth, w_lon, b_lon, w_lat, b_lat, reps: int = 1,
           **run_kwargs):
    mm, wts = _pack_consts(w_depth, b_depth, w_lon, b_lon, w_lat, b_lat)
    xf = np.ascontiguousarray(np.asarray(x), dtype=np.float32).reshape(N_CORES, CORE_ELEMS)
    in_maps = [{"x": xf[c], "mm": mm, "wts": wts} for c in range(N_CORES)]
    nc = _get_nc(reps)
    res = run_bass_kernel_spmd(nc, in_maps, core_ids=list(range(N_CORES)), **run_kwargs)
    out = np.stack([r["y"] for r in res.results], axis=0)
    out = out.reshape(B, 15, 64, 128, 1)
    if run_kwargs:
        kernel.last_results = res
    return out


# revision 19
# speedup vs baseline: 1.1043x; 1.1043x over previous
"""Trainium2 Bass kernel for nn_LocalNetwork (avgpool3d -> 3x LocallyConnected1D -> upsample3d).

Sharding: pure data parallelism — batch 256 split as 32 per core across 8 cores.

Per-core layout (B_loc=32 batches, 4 load groups of 8, conv pairs of 2 groups):
  partition p = (bl, dslice)  [8 x 15 = 120 partitions]
  Every DMA descriptor covers a 32KB-contiguous DRAM run (one (h,w) slice).

  - avg-pool over (ws) then (hs): 3+3 tensor_tensor adds (DVE runs TT at
    ~2.4 elem/cycle vs 1 for tensor_reduce)
  - depth pool + depth-conv taps: matmuls [120 -> 40] with 0/(1/48)
    matrices (fuses the /48 mean scale and the +/-1 depth shifts)
  - two load-groups pair into [80, 512] conv tiles so the ~680ns/instr
    DVE overhead is amortized over 16 batches
  - upsample: relu + h-expand into ws=0 lanes of U (scalar engine),
    w-expand ws=1..3 in place (vector, broadcast src); depth x3
    replication is FREE — three store DMAs per group read the same
    [40, 8192] partition-slice of U into interleaved DRAM depth slices.
  - loads issue on the sync HWDGE queue, stores on the scalar (Act)
    HWDGE queue: a store waiting for compute must not head-of-line
    block later loads.
"""

import numpy as np

import concourse.bass as bass
import concourse.mybir as mybir
from concourse import bacc
from concourse.bass_utils import run_bass_kernel_spmd
from concourse.tile import TileContext

F32 = mybir.dt.float32
ADD = mybir.AluOpType.add
MULT = mybir.AluOpType.mult
RELU = mybir.ActivationFunctionType.Relu

N_CORES = 8
B = 256
B_CORE = 32          # batches per core
G = 4                # load groups per core
B_GRP = 8            # batches per group
CORE_ELEMS = B_CORE * 15 * 64 * 128  # 3,932,160
BSTRIDE = 15 * 64 * 128              # 122,880
SLICE = 64 * 128                     # 8192 elems = one (h,w) plane = 32KB


def _pack_consts(w_depth, b_depth, w_lon, b_lon, w_lat, b_lat):
    """Returns (mm [120,128] f32, wts [104,6144] f32).

    mm: three matmul lhsT tiles [120,40] (cols 0:40 dn / 40:80 mid / 80:120 up)
        out[q=(bl,dp), f] = sum_p lhsT[p=(bl,dsl), q] * P2[p, f]
        coefficient 1/48 folds the avg-pool mean.
    wts: 12 x [40,512] conv weight/bias tiles, p=(bl,dp), f=(ho,wo).
    """
    mm = np.zeros((120, 128), np.float32)
    for bl in range(8):
        for dsl in range(15):
            p = bl * 15 + dsl
            grp = dsl // 3
            for col0, dp in ((0, grp + 1), (40, grp), (80, grp - 1)):
                if 0 <= dp <= 4:
                    mm[p, col0 + bl * 5 + dp] = 1.0 / 48.0

    dp = np.arange(5)[:, None, None]
    ho = np.arange(16)[None, :, None]
    wo = np.arange(32)[None, None, :]
    ld = wo * 112 + ho * 7 + (dp + 1)     # depth seq index (5,16,32)
    ll = dp * 544 + ho * 34 + (wo + 1)    # lon
    lt = dp * 576 + wo * 18 + (ho + 1)    # lat

    def tile(vec, idx):
        t = np.broadcast_to(np.asarray(vec)[idx][None], (8, 5, 16, 32))
        return t.reshape(40, 512)

    cols = []
    for j in range(3):
        cols.append(tile(np.asarray(w_depth)[:, j], ld))
    cols.append(tile(b_depth, ld))
    for j in range(3):
        cols.append(tile(np.asarray(w_lon)[:, j], ll))
    cols.append(tile(b_lon, ll))
    for j in range(3):
        cols.append(tile(np.asarray(w_lat)[:, j], lt))
    cols.append(tile(b_lat, lt))
    wts40 = np.concatenate(cols, axis=1)
    wts = np.zeros((104, 6144), np.float32)
    wts[0:40] = wts40
    wts[64:104] = wts40
    return mm, np.ascontiguousarray(wts)


def build_nc(reps: int = 1) -> bass.Bass:
    nc = bacc.Bacc("TRN2", target_bir_lowering=False, debug=False)
    x = nc.dram_tensor("x", [CORE_ELEMS], F32, kind="ExternalInput")
    mmc = nc.dram_tensor("mm", [120, 128], F32, kind="ExternalInput")
    wtc = nc.dram_tensor("wts", [104, 6144], F32, kind="ExternalInput")
    y = nc.dram_tensor("y", [CORE_ELEMS], F32, kind="ExternalOutput")

    with TileContext(nc) as tc:
        with (
            tc.tile_pool(name="cpool", bufs=1) as cpool,
            tc.tile_pool(name="inp", bufs=2) as inp,
            tc.tile_pool(name="outp", bufs=2) as outp,
            tc.tile_pool(name="work", bufs=2) as work,
            tc.tile_pool(name="psum", bufs=2, space="PSUM") as psum,
        ):
            MM = cpool.tile([120, 128], F32)
            WT = cpool.tile([104, 6144], F32)
            nc.sync.dma_start(MM[:], mmc[:])
            nc.sync.dma_start(WT[:], wtc[:])
            w = lambda i: WT[:, i * 512:(i + 1) * 512]
            wd0, wd1, wd2, bd = (w(i) for i in range(4))
            vl0, vl1, vl2, blon = (w(i) for i in range(4, 8))
            ul0, ul1, ul2, blat = (w(i) for i in range(8, 12))

            state = {}

            def load(g, pieces=2):
                # split loads so the first piece's pool reduce overlaps the
                # rest of the transfer (and the first group primes fast)
                off = (g % G) * B_GRP * BSTRIDE
                X = inp.tile([120, SLICE], F32)
                step = SLICE // pieces
                for c in range(pieces):
                    nc.sync.dma_start(
                        X[:, c * step:(c + 1) * step],
                        bass.AP(x, off + c * step,
                                [[BSTRIDE, 8], [SLICE, 15], [1, step]]))
                state[g] = X

            def pool(g):
                X = state.pop(g)
                # h,w avg-pool (sum): fused reduce over (hs, ws), one per
                # load half (ho 0:8 | 8:16)
                P2 = work.tile([120, 512], F32)
                for c in range(2):
                    nc.vector.tensor_reduce(
                        P2[:, c * 256:(c + 1) * 256]
                            .rearrange("p (ho wo) -> p ho wo", ho=8),
                        X[:, c * 4096:(c + 1) * 4096]
                            .rearrange("p (ho hs wo ws) -> p ho wo hs ws",
                                       ho=8, hs=4, wo=32, ws=4),
                        mybir.AxisListType.XY, ADD)
                state[("P2", g)] = P2

            def mm(g):
                # depth pool (/48) + conv taps; pair half a at partitions
                # 0:40, half b at 64:104 (matmul out base must be 0/32/64)
                k, half = divmod(g, 2)
                if half == 0:
                    Sdn = psum.tile([104, 512], F32)
                    S0 = psum.tile([104, 512], F32)
                    Sup = psum.tile([104, 512], F32)
                    state[("S", k)] = (Sdn, S0, Sup)
                else:
                    Sdn, S0, Sup = state[("S", k)]
                P2 = state.pop(("P2", g))
                sl = slice(64 * half, 64 * half + 40)
                nc.tensor.matmul(Sdn[sl], MM[:, 0:40], P2[:], start=True, stop=True)
                nc.tensor.matmul(S0[sl], MM[:, 40:80], P2[:], start=True, stop=True)
                nc.tensor.matmul(Sup[sl], MM[:, 80:120], P2[:], start=True, stop=True)

            def conv_store_pair(k):
                ga, gb = 2 * k, 2 * k + 1
                Sdn, S0, Sup = state.pop(("S", k))
                # depth conv: 3 independent mults, then the add chain
                m = work.tile([104, 512], F32)
                m2 = work.tile([104, 512], F32)
                m3 = work.tile([104, 512], F32)
                nc.vector.tensor_tensor(m[:], wd0, Sdn[:], MULT)
                nc.vector.tensor_tensor(m2[:], wd1, S0[:], MULT)
                nc.vector.tensor_tensor(m3[:], wd2, Sup[:], MULT)
                nc.vector.tensor_tensor(m[:], m[:], m2[:], ADD)
                nc.vector.tensor_tensor(m[:], m[:], m3[:], ADD)
                nc.vector.tensor_tensor(m[:], m[:], bd, ADD)
                # relu into lon-padded tile Ydp[p, ho*34 + (wo+1)]
                Ydp = work.tile([104, 544], F32)
                Ydpv = Ydp[:].rearrange("p (ho wp) -> p ho wp", ho=16, wp=34)
                nc.gpsimd.memset(Ydpv[:, :, 0], 0)
                nc.gpsimd.memset(Ydpv[:, :, 33], 0)
                nc.vector.tensor_scalar_max(
                    Ydpv[:, :, 1:33],
                    m[:].rearrange("p (ho wo) -> p ho wo", ho=16), 0.0)

                # lon conv (along wo, free axis)
                m3v = m3[:].rearrange("p (ho wo) -> p ho wo", ho=16)
                mv = m[:].rearrange("p (ho wo) -> p ho wo", ho=16)
                m2v = m2[:].rearrange("p (ho wo) -> p ho wo", ho=16)
                w3 = lambda t: t.rearrange("p (ho wo) -> p ho wo", ho=16)
                nc.vector.tensor_tensor(mv, w3(vl0), Ydpv[:, :, 0:32], MULT)
                nc.vector.tensor_tensor(m2v, w3(vl1), Ydpv[:, :, 1:33], MULT)
                nc.vector.tensor_tensor(m3v, w3(vl2), Ydpv[:, :, 2:34], MULT)
                nc.vector.tensor_tensor(mv, mv, m2v, ADD)
                nc.vector.tensor_tensor(mv, mv, m3v, ADD)
                nc.vector.tensor_tensor(mv, mv, w3(blon), ADD)
                # relu into lat-padded tile Ylp[p, (ho+1)*32 + wo]
                Ylp = work.tile([104, 576], F32)
                nc.gpsimd.memset(Ylp[:, 0:32], 0)
                nc.gpsimd.memset(Ylp[:, 544:576], 0)
                nc.vector.tensor_scalar_max(Ylp[:, 32:544], m[:], 0.0)

                # lat conv (along ho, free axis; contiguous slices)
                nc.vector.tensor_tensor(m[:], ul0, Ylp[:, 0:512], MULT)
                nc.vector.tensor_tensor(m2[:], ul1, Ylp[:, 32:544], MULT)
                nc.vector.tensor_tensor(m3[:], ul2, Ylp[:, 64:576], MULT)
                nc.vector.tensor_tensor(m[:], m[:], m2[:], ADD)
                nc.vector.tensor_tensor(m[:], m[:], m3[:], ADD)
                nc.vector.tensor_tensor(m[:], m[:], blat, ADD)

                # upsample: relu + h-expand (scalar), w-expand (vector)
                A = work.tile([104, 2048], F32)  # (ho, hs, wo)
                Av = A[:].rearrange("p (ho hs wo) -> p ho hs wo", ho=16, hs=4)
                mb = m[:].rearrange("p (ho wo) -> p ho wo", ho=16) \
                         .unsqueeze(2).broadcast_to([104, 16, 4, 32])
                nc.scalar.activation(Av, mb, RELU)
                U = outp.tile([104, SLICE], F32)  # (h, wo, ws)
                Uw = U[:].rearrange("p (h wo ws) -> p h wo ws", h=64, ws=4)
                Ab = A[:].rearrange("p (h wo) -> p h wo", h=64) \
                         .unsqueeze(3).broadcast_to([104, 64, 32, 4])
                nc.vector.tensor_scalar_add(Uw, Ab, 0.0)

                # stores alternate queues per di (all stores are emitted
                # after all loads, so no head-of-line blocking on q1)
                for half, g in enumerate((ga, gb)):
                    off = (g % G) * B_GRP * BSTRIDE
                    for di in range(3):
                        eng = nc.scalar if half == 0 else nc.sync
                        eng.dma_start(
                            bass.AP(y, off + di * SLICE,
                                    [[BSTRIDE, 8], [3 * SLICE, 5], [1, SLICE]]),
                            U[64 * half:64 * half + 40, :])

            def conv_store_single(g):
                Sdn, S0, Sup = state.pop(("S1", g))
                w40 = lambda i: WT[0:40, i * 512:(i + 1) * 512]
                m = work.tile([40, 512], F32)
                m2 = work.tile([40, 512], F32)
                m3 = work.tile([40, 512], F32)
                nc.vector.tensor_tensor(m[:], w40(0), Sdn[:], MULT)
                nc.vector.tensor_tensor(m2[:], w40(1), S0[:], MULT)
                nc.vector.tensor_tensor(m3[:], w40(2), Sup[:], MULT)
                nc.vector.tensor_tensor(m[:], m[:], m2[:], ADD)
                nc.vector.tensor_tensor(m[:], m[:], m3[:], ADD)
                nc.vector.tensor_tensor(m[:], m[:], w40(3), ADD)
                Ydp = work.tile([40, 544], F32)
                Ydpv = Ydp[:].rearrange("p (ho wp) -> p ho wp", ho=16, wp=34)
                nc.gpsimd.memset(Ydpv[:, :, 0], 0)
                nc.gpsimd.memset(Ydpv[:, :, 33], 0)
                nc.vector.tensor_scalar_max(
                    Ydpv[:, :, 1:33],
                    m[:].rearrange("p (ho wo) -> p ho wo", ho=16), 0.0)
                mv = m[:].rearrange("p (ho wo) -> p ho wo", ho=16)
                m2v = m2[:].rearrange("p (ho wo) -> p ho wo", ho=16)
                m3v = m3[:].rearrange("p (ho wo) -> p ho wo", ho=16)
                w340 = lambda i: w40(i).rearrange("p (ho wo) -> p ho wo", ho=16)
                nc.vector.tensor_tensor(mv, w340(4), Ydpv[:, :, 0:32], MULT)
                nc.vector.tensor_tensor(m2v, w340(5), Ydpv[:, :, 1:33], MULT)
                nc.vector.tensor_tensor(m3v, w340(6), Ydpv[:, :, 2:34], MULT)
                nc.vector.tensor_tensor(mv, mv, m2v, ADD)
                nc.vector.tensor_tensor(mv, mv, m3v, ADD)
                nc.vector.tensor_tensor(mv, mv, w340(7), ADD)
                Ylp = work.tile([40, 576], F32)
                nc.gpsimd.memset(Ylp[:, 0:32], 0)
                nc.gpsimd.memset(Ylp[:, 544:576], 0)
                nc.vector.tensor_scalar_max(Ylp[:, 32:544], m[:], 0.0)
                nc.vector.tensor_tensor(m[:], w40(8), Ylp[:, 0:512], MULT)
                nc.vector.tensor_tensor(m2[:], w40(9), Ylp[:, 32:544], MULT)
                nc.vector.tensor_tensor(m3[:], w40(10), Ylp[:, 64:576], MULT)
                nc.vector.tensor_tensor(m[:], m[:], m2[:], ADD)
                nc.vector.tensor_tensor(m[:], m[:], m3[:], ADD)
                nc.vector.tensor_tensor(m[:], m[:], w40(11), ADD)
                A = work.tile([40, 2048], F32)
                Av = A[:].rearrange("p (ho hs wo) -> p ho hs wo", ho=16, hs=4)
                mb = m[:].rearrange("p (ho wo) -> p ho wo", ho=16) \
                         .unsqueeze(2).broadcast_to([40, 16, 4, 32])
                nc.scalar.activation(Av, mb, RELU)
                U = outp.tile([40, SLICE], F32)
                Uw = U[:].rearrange("p (h wo ws) -> p h wo ws", h=64, ws=4)
                Ab = A[:].rearrange("p (h wo) -> p h wo", h=64) \
                         .unsqueeze(3).broadcast_to([40, 64, 32, 4])
                nc.vector.tensor_scalar_add(Uw, Ab, 0.0)
                off = (g % G) * B_GRP * BSTRIDE
                for di in range(3):
                    eng = nc.scalar if (g + di) % 2 else nc.sync
                    eng.dma_start(
                        bass.AP(y, off + di * SLICE,
                                [[BSTRIDE, 8], [3 * SLICE, 5], [1, SLICE]]),
                        U[:])

            # software-pipelined emission
            for r in range(reps):
                b = r * G
                load(b + 0, pieces=4)
                load(b + 1)
                pool(b + 0)
                mm(b + 0)
                load(b + 2)
                pool(b + 1)
                mm(b + 1)
                load(b + 3)
                conv_store_pair(b // 2 + 0)
                with tc.tile_wait_until(0.072 + r * 0.2):
                    pool(b + 2)
                    mm(b + 2)
                with tc.tile_wait_until(0.082 + r * 0.2):
                    pool(b + 3)
                    mm(b + 3)
                conv_store_pair(b // 2 + 1)

    nc.compile()
    return nc


_NC_CACHE = {}


def _get_nc(reps: int = 1):
    if reps not in _NC_CACHE:
        _NC_CACHE[reps] = build_nc(reps)
    return _NC_CACHE[reps]


def kernel(x, w_depth, b_depth, w_lon, b_lon, w_lat, b_lat, reps: int = 1,
           **run_kwargs):
    mm, wts = _pack_consts(w_depth, b_depth, w_lon, b_lon, w_lat, b_lat)
    xf = np.ascontiguousarray(np.asarray(x), dtype=np.float32).reshape(N_CORES, CORE_ELEMS)
    in_maps = [{"x": xf[c], "mm": mm, "wts": wts} for c in range(N_CORES)]
    nc = _get_nc(reps)
    res = run_bass_kernel_spmd(nc, in_maps, core_ids=list(range(N_CORES)), **run_kwargs)
    out = np.stack([r["y"] for r in res.results], axis=0)
    out = out.reshape(B, 15, 64, 128, 1)
    if run_kwargs:
        kernel.last_results = res
    return out
